# revision 12
# baseline (speedup 1.0000x reference)
"""T5-style causal multi-head attention (B=4, S=2048, E=1024, H=16, D=64)
on 8 NeuronCores. Sharding: core c handles batch c//2 and head half c%2
(8 heads). Host sums the two row-parallel partial output projections per
batch.

v3:
  - host ships x^T (bf16) so stage 1 is pure projection matmuls.
  - multiplicative bias: P = exp(QK/8) * exp(bias); near-diagonal tiles
    multiply by a precomputed exp(bias)&mask table on DVE; far tiles
    (distance >= 513, bucket saturated at 31) use a pre-scaled copy of V.
  - fully-masked columns of near-diagonal tiles are skipped.
  - stage 2 is software-pipelined: PV matmuls trail their QK matmuls by
    LAG iterations so the PE never waits on the exp/bias of the tile it
    just produced; the pipeline runs flat across all (hp, qc, kb).
  - per-head-pair softmax reciprocal via reciprocal_approx_fast on DVE;
    1/den broadcast across partitions with tiny bf16 K=1 PE matmuls;
    normalization of head-pair hp is deferred into hp+1's stream.
  - oT / Wo in bf16 for a faster output projection.
"""
import sys

sys.path.insert(0, "/opt/trn_rl_repo")

import numpy as np
import ml_dtypes

import concourse.bass as bass
import concourse.mybir as mybir
import concourse.tile as tile
from concourse import bacc
from concourse.bass_utils import run_bass_kernel_spmd

F32, F32R, BF16 = mybir.dt.float32, mybir.dt.float32r, mybir.dt.bfloat16
AF = mybir.ActivationFunctionType
MULT = mybir.AluOpType.mult

B, S, E, H, D = 4, 2048, 1024, 16, 64
HL = H // 2          # heads per core
HD = HL * D          # 512, per-core head dims
NUM_BUCKETS, MAX_DISTANCE = 32, 128
NT = S // 128        # 16 token blocks
NE = E // 128        # 8 embed chunks
NFAR = 8             # far tiles only occur for kb < 8

_NC_CACHE = {}


# ---------------------------------------------------------------- host side

def _np_bucket(distance):
    """Mirror reference._relative_position_bucket for causal (distance>=0),
    float32 arithmetic like jnp."""
    max_exact = NUM_BUCKETS // 2  # 16
    is_small = distance < max_exact
    safe = np.maximum(distance, 1).astype(np.float32)
    log_scale = np.log(safe / np.float32(max_exact)).astype(np.float32) / np.float32(
        np.log(np.float32(MAX_DISTANCE / max_exact))
    )
    large = max_exact + (log_scale * np.float32(NUM_BUCKETS - max_exact)).astype(
        np.int32
    )
    large = np.minimum(large, NUM_BUCKETS - 1)
    return np.where(is_small, distance, large)


def _build_ebtab(rel_bias_half):
    """rel_bias_half [8, 32] -> ebtab [4 hp, 128 k, 8 mi, 2 h, 512 q] bf16
    holding exp(bias) * causal_mask (0/1). mi = (4*qc - kb) + 3 in [0, 8)."""
    qq = np.arange(512)[None, :]
    kk = np.arange(128)[:, None]
    eb = np.exp(rel_bias_half.astype(np.float32))  # [8, 32]
    tiles = []
    for mi in range(8):
        m = mi - 3
        dd = 128 * m + qq - kk  # q - k distance, [128, 512]
        bucket = _np_bucket(np.maximum(dd, 0))
        vals = eb[:, bucket]  # [8, 128, 512]
        vals = np.where(dd[None] >= 0, vals, 0.0).astype(np.float32)
        tiles.append(vals)
    t = np.stack(tiles, axis=0)  # [8 mi, 8 h, 128 k, 512 q]
    t = t.reshape(8, 4, 2, 128, 512).transpose(1, 3, 0, 2, 4)
    return np.ascontiguousarray(t).astype(ml_dtypes.bfloat16)


def _make_in_maps(inputs_q, inputs_kv, Wq, Wk, Wv, Wo, rel_bias):
    bf = ml_dtypes.bfloat16
    ebtabs = [_build_ebtab(rel_bias[0:HL]), _build_ebtab(rel_bias[HL:])]
    cvecs = []
    for half in range(2):
        c = np.exp(rel_bias[half * HL:(half + 1) * HL, 31].astype(np.float32))
        cv = np.broadcast_to(np.repeat(c, 65)[None, :], (128, HL * 65))
        cvecs.append(np.ascontiguousarray(cv).astype(bf))
    xqTs = [np.ascontiguousarray(inputs_q[b].T.astype(bf)) for b in range(B)]
    xkvTs = [np.ascontiguousarray(inputs_kv[b].T.astype(bf)) for b in range(B)]
    wqs, wks, wvs, wos = [], [], [], []
    for half in range(2):
        sl = slice(half * HD, (half + 1) * HD)
        wqs.append(np.ascontiguousarray(Wq[:, sl].astype(bf)))
        wks.append(np.ascontiguousarray(Wk[:, sl].astype(bf)))
        wvs.append(np.ascontiguousarray(Wv[:, sl].astype(bf)))
        wos.append(np.ascontiguousarray(Wo[sl, :].astype(bf)))
    in_maps = []
    for c in range(8):
        b, half = c // 2, c % 2
        in_maps.append({
            "xqT": xqTs[b],
            "xkvT": xkvTs[b],
            "wq": wqs[half],
            "wk": wks[half],
            "wv": wvs[half],
            "wo": wos[half],
            "ebtab": ebtabs[half],
            "cvec": cvecs[half],
        })
    return in_maps


# -------------------------------------------------------------- device side

def _build_nc():
    nc = bacc.Bacc(None, target_bir_lowering=False)
    xqT_d = nc.dram_tensor("xqT", [E, S], BF16, kind="ExternalInput")
    xkvT_d = nc.dram_tensor("xkvT", [E, S], BF16, kind="ExternalInput")
    wq_d = nc.dram_tensor("wq", [E, HD], BF16, kind="ExternalInput")
    wk_d = nc.dram_tensor("wk", [E, HD], BF16, kind="ExternalInput")
    wv_d = nc.dram_tensor("wv", [E, HD], BF16, kind="ExternalInput")
    wo_d = nc.dram_tensor("wo", [HD, E], BF16, kind="ExternalInput")
    eb_d = nc.dram_tensor("ebtab", [4, 128, 8, 2, 512], BF16,
                          kind="ExternalInput")
    cv_d = nc.dram_tensor("cvec", [128, HL * 65], BF16, kind="ExternalInput")
    out_d = nc.dram_tensor("out", [S, E], F32, kind="ExternalOutput")

    with tile.TileContext(nc) as tc:
        with (
            tc.tile_pool(name="const", bufs=1) as pconst,
            tc.tile_pool(name="persist", bufs=1) as pper,
        ):
            onesb = pconst.tile([128, 64], BF16)
            nc.vector.memset(onesb, 1.0)
            cvec = pconst.tile([128, HL * 65], BF16)
            eb0 = pconst.tile([128, 8, 2, 512], BF16)

            qT = pper.tile([128, 4, S], F32R)         # [pair-dims, hc, tok]
            kT = pper.tile([128, 4, S], F32R)
            vA = pper.tile([128, NT, HL * 65], BF16)  # v + ones col per head
            # vC only feeds far tiles (kb <= 4*qc-5 <= 7)
            vC = pper.tile([128, NFAR, HL * 65], BF16)  # vA * exp(b31_h)
            # per-head-pair denominator rows: head hh -> partition 32*hh
            den2 = pper.tile([128, 4, 512], F32)
            denRb = pper.tile([128, 4, 512], BF16)
            denS = pper.tile([128, 2, 32], F32)   # [64, head, 32] reshape

            vAr = vA.rearrange("p t (h c) -> p t h c", c=65)
            nc.vector.memset(vAr[:, :, :, 64:65], 1.0)

            # ---------------- stage 1: projections (x^T comes from host)
            with tc.tile_pool(name="s1x", bufs=1) as p1x:
                xkv_sb = p1x.tile([128, NE, S], BF16)
                xq_sb = p1x.tile([128, NE, S], BF16)

                with (
                    tc.tile_pool(name="s1wv", bufs=1) as p1wv,
                    tc.tile_pool(name="psPv", bufs=4, space="PSUM") as psPv,
                ):
                    wv_sb = p1wv.tile([128, NE, HD], BF16)
                    nc.sync.dma_start(
                        out=wv_sb,
                        in_=wv_d[:].rearrange("(c p) n -> p c n", p=128))
                    for hv in range(2):
                        for e in range(NE):
                            nc.sync.dma_start(
                                out=xkv_sb[:, e, hv * 1024:(hv + 1) * 1024],
                                in_=xkvT_d[e * 128:(e + 1) * 128,
                                           hv * 1024:(hv + 1) * 1024])
                    for hv in range(2):
                        for e in range(NE):
                            nc.sync.dma_start(
                                out=xq_sb[:, e, hv * 1024:(hv + 1) * 1024],
                                in_=xqT_d[e * 128:(e + 1) * 128,
                                          hv * 1024:(hv + 1) * 1024])
                    nc.sync.dma_start(out=cvec, in_=cv_d[:])
                    nc.sync.dma_start(out=eb0, in_=eb_d[0])
                    # v projection: out [tok, hd] blocks, evicted in pairs
                    for tp in range(NT // 2):
                        vps = psPv.tile([128, 2, HD], F32, tag="pj")
                        for ti in range(2):
                            t = 2 * tp + ti
                            for e in range(NE):
                                nc.tensor.matmul(
                                    vps[:, ti, :],
                                    xkv_sb[:, e, t * 128:(t + 1) * 128],
                                    wv_sb[:, e, :],
                                    start=(e == 0), stop=(e == NE - 1))
                        nc.vector.tensor_copy(
                            vAr[:, 2 * tp:2 * tp + 2, :, 0:64],
                            vps.rearrange("p a (h c) -> p a h c", c=64))
                    for t in range(NFAR):
                        nc.vector.tensor_tensor(
                            out=vC[:, t, :], in0=vA[:, t, :], in1=cvec,
                            op=MULT)

                # q/k projections: out qT/kT [2-head dims, tok]
                with (
                    tc.tile_pool(name="s1wqk", bufs=1) as p1w,
                    tc.tile_pool(name="psP", bufs=2, space="PSUM") as psP,
                ):
                    wk_sb = p1w.tile([128, NE, HD], BF16)
                    wq_sb = p1w.tile([128, NE, HD], BF16)
                    for w_sb, w_dr in ((wk_sb, wk_d), (wq_sb, wq_d)):
                        nc.sync.dma_start(
                            out=w_sb,
                            in_=w_dr[:].rearrange("(c p) n -> p c n", p=128))
                    for w_sb, x_sb, dst in ((wk_sb, xkv_sb, kT),
                                            (wq_sb, xq_sb, qT)):
                        for hc in range(4):
                            ps = psP.tile([128, 4, 512], F32, tag="pj")
                            for e in range(NE):
                                for tq in range(4):
                                    nc.tensor.matmul(
                                        ps[:, tq, :],
                                        w_sb[:, e, hc * 128:(hc + 1) * 128],
                                        x_sb[:, e, tq * 512:(tq + 1) * 512],
                                        start=(e == 0), stop=(e == NE - 1))
                            nc.vector.tensor_copy(
                                dst[:, hc, :],
                                ps.rearrange("p a b -> p (a b)"))

            # ---------------- stages 2+3 share the O^T pool
            with (
                tc.tile_pool(name="persist2", bufs=1) as pper2,
                tc.tile_pool(name="s3w", bufs=1) as p3w,
            ):
                oT = pper2.tile([128, 4, S], BF16)
                wo_sb = p3w.tile([128, 4, E], BF16)
                nc.sync.dma_start(
                    out=wo_sb,
                    in_=wo_d[:].rearrange("(g p) n -> p g n", p=128))

                with (
                    tc.tile_pool(name="s2eb", bufs=2) as p2b,
                    tc.tile_pool(name="s2p", bufs=6) as p2p,
                    tc.tile_pool(name="psS", bufs=3, space="PSUM") as psS,
                    tc.tile_pool(name="psO", bufs=1, space="PSUM") as psO,
                ):
                    eb_tiles = {0: eb0}
                    iters = [(hp, qc, kb)
                             for hp in range(4)
                             for qc in range(4)
                             for kb in range(4 * qc + 4)]
                    LAG = 3
                    p_info = {}
                    o_cur = [None]
                    norm_pending = []

                    def emit_norm_qc(hp, qc):
                        pr = psS.tile([128, 512], F32, tag="s",
                                      name=f"pr{hp}_{qc}")
                        for hh in range(2):
                            bp = 32 * hh
                            nc.tensor.matmul(
                                pr[64 * hh:64 * hh + 64, :],
                                onesb[bp:bp + 1, 0:64],
                                denRb[bp:bp + 1, qc, :],
                                start=True, stop=True)
                        sl = oT[:, hp, qc * 512:(qc + 1) * 512]
                        nc.vector.tensor_tensor(
                            out=sl, in0=sl, in1=pr, op=MULT)

                    def emit_norm(hp):
                        for qc in range(4):
                            emit_norm_qc(hp, qc)

                    for j in range(len(iters) + LAG):
                        if j < len(iters):
                            hp, qc, kb = iters[j]
                            if qc == 1 and kb == 0 and hp < 3:
                                ebn = p2b.tile([128, 8, 2, 512], BF16,
                                               tag="eb", name=f"eb{hp + 1}")
                                nc.sync.dma_start(out=ebn, in_=eb_d[hp + 1])
                                eb_tiles[hp + 1] = ebn
                            m = 4 * qc - kb
                            q0 = 0 if m >= 0 else 128 * min(-m, 2)
                            s = psS.tile([128, 2, 512], F32, tag="s",
                                         name=f"s{j}")
                            nc.tensor.matmul(
                                s[:, 0, q0:512],
                                kT[0:64, hp, kb * 128:(kb + 1) * 128],
                                qT[0:64, hp, qc * 512 + q0:(qc + 1) * 512],
                                start=True, stop=True)
                            nc.tensor.matmul(
                                s[:, 1, q0:512],
                                kT[64:128, hp, kb * 128:(kb + 1) * 128],
                                qT[64:128, hp, qc * 512 + q0:(qc + 1) * 512],
                                start=True, stop=True)
                            p = p2p.tile([128, 2, 512], BF16, tag="p",
                                         name=f"p{j}")
                            nc.scalar.activation(
                                p[:, :, q0:512], s[:, :, q0:512],
                                AF.Exp, scale=0.125)
                            if m <= 4:  # near diagonal: elementwise bias
                                nc.vector.tensor_tensor(
                                    out=p[:, :, q0:512],
                                    in0=p[:, :, q0:512],
                                    in1=eb_tiles[hp][:, m + 3, :, q0:512],
                                    op=MULT)
                                p_info[j] = (p, q0, vA)
                            else:       # far: bias folded into vC
                                p_info[j] = (p, q0, vC)
                        jj = j - LAG
                        if jj >= 0:
                            hp, qc, kb = iters[jj]
                            h0, h1 = 2 * hp, 2 * hp + 1
                            p, q0, vsrc = p_info.pop(jj)
                            nkb = 4 * qc + 4
                            if kb == 0:
                                o_cur[0] = psO.tile([65, 2, 512], F32,
                                                    tag="o",
                                                    name=f"o{hp}_{qc}")
                            o = o_cur[0]
                            nc.tensor.matmul(
                                o[:, 0, q0:512],
                                vsrc[:, kb, h0 * 65:(h0 + 1) * 65],
                                p[:, 0, q0:512],
                                start=(kb == 0), stop=(kb == nkb - 1),
                                skip_group_check=True)
                            nc.tensor.matmul(
                                o[:, 1, q0:512],
                                vsrc[:, kb, h1 * 65:(h1 + 1) * 65],
                                p[:, 1, q0:512],
                                start=(kb == 0), stop=(kb == nkb - 1),
                                skip_group_check=True)
                            if kb == nkb - 1:
                                # epilogue: stash raw O^T + denominators
                                for hh in range(2):
                                    bp = 32 * hh
                                    nc.vector.tensor_copy(
                                        den2[bp:bp + 1, qc, :],
                                        o[64:65, hh, :])
                                    nc.vector.tensor_copy(
                                        oT[64 * hh:64 * (hh + 1), hp,
                                           qc * 512:(qc + 1) * 512],
                                        o[0:64, hh, :])
                                if hp == 3:
                                    # deferred norm for hp=2 must read denRb
                                    # before this qc's chain overwrites it
                                    if qc == 0 and norm_pending:
                                        emit_norm(norm_pending.pop(0))
                                    # eager per-qc recip + norm for last hp
                                    for hh in range(2):
                                        bp = 32 * hh
                                        nc.sync.dma_start(
                                            out=denS[0:16, hh, :],
                                            in_=den2[bp:bp + 1, qc, :])
                                        nc.vector.reciprocal(
                                            denS[0:16, hh, :],
                                            denS[0:16, hh, :])
                                        nc.sync.dma_start(
                                            out=den2[bp:bp + 1, qc, :],
                                            in_=denS[0:16, hh, :])
                                        nc.vector.tensor_copy(
                                            denRb[bp:bp + 1, qc, :],
                                            den2[bp:bp + 1, qc, :])
                                    emit_norm_qc(3, qc)
                                elif qc == 3:
                                    for hh in range(2):
                                        bp = 32 * hh
                                        nc.sync.dma_start(
                                            out=denS[0:64, hh, :],
                                            in_=den2[bp:bp + 1, :, :])
                                        nc.vector.reciprocal(
                                            denS[0:64, hh, :],
                                            denS[0:64, hh, :])
                                        nc.sync.dma_start(
                                            out=den2[bp:bp + 1, :, :],
                                            in_=denS[0:64, hh, :])
                                        nc.vector.tensor_copy(
                                            denRb[bp:bp + 1, :, :],
                                            den2[bp:bp + 1, :, :])
                                    norm_pending.append(hp)
                                elif qc == 0 and norm_pending:
                                    emit_norm(norm_pending.pop(0))
                    while norm_pending:
                        emit_norm(norm_pending.pop(0))

                # ---------------- stage 3: output projection
                with (
                    tc.tile_pool(name="s3o", bufs=3) as p3o,
                    tc.tile_pool(name="psF", bufs=4, space="PSUM") as psF,
                ):
                    for t in range(NT):
                        oev = p3o.tile([128, E], F32, tag="oev")
                        ops = [psF.tile([128, 512], F32, tag="ops",
                                        name=f"ops{ec}")
                               for ec in range(2)]
                        for hp in range(4):
                            for ec in range(2):
                                nc.tensor.matmul(
                                    ops[ec], oT[:, hp, t * 128:(t + 1) * 128],
                                    wo_sb[:, hp, ec * 512:(ec + 1) * 512],
                                    start=(hp == 0), stop=(hp == 3))
                        for ec in range(2):
                            nc.vector.tensor_copy(
                                oev[:, ec * 512:(ec + 1) * 512], ops[ec])
                        nc.sync.dma_start(
                            out=out_d[t * 128:(t + 1) * 128, :], in_=oev)

    nc.compile()
    return nc


def _get_nc():
    if "nc" not in _NC_CACHE:
        _NC_CACHE["nc"] = _build_nc()
    return _NC_CACHE["nc"]


def kernel(inputs_q, inputs_kv, mask, Wq, Wk, Wv, Wo, rel_bias):
    inputs_q = np.asarray(inputs_q, dtype=np.float32)
    inputs_kv = np.asarray(inputs_kv, dtype=np.float32)
    Wq = np.asarray(Wq, dtype=np.float32)
    Wk = np.asarray(Wk, dtype=np.float32)
    Wv = np.asarray(Wv, dtype=np.float32)
    Wo = np.asarray(Wo, dtype=np.float32)
    rel_bias = np.asarray(rel_bias, dtype=np.float32)

    nc = _get_nc()
    in_maps = _make_in_maps(inputs_q, inputs_kv, Wq, Wk, Wv, Wo, rel_bias)
    res = run_bass_kernel_spmd(nc, in_maps, core_ids=list(range(8)))
    out = np.stack(
        [res.results[2 * b]["out"] + res.results[2 * b + 1]["out"]
         for b in range(B)])
    return out.astype(np.float32)


# revision 13
# speedup vs baseline: 1.0238x; 1.0238x over previous
"""T5-style causal multi-head attention (B=4, S=2048, E=1024, H=16, D=64)
on 8 NeuronCores. Sharding: core c handles batch c//2 and head half c%2
(8 heads). Host sums the two row-parallel partial output projections per
batch.

v3:
  - host ships x^T (bf16) so stage 1 is pure projection matmuls.
  - multiplicative bias: P = exp(QK/8) * exp(bias); near-diagonal tiles
    multiply by a precomputed exp(bias)&mask table on DVE; far tiles
    (distance >= 513, bucket saturated at 31) use a pre-scaled copy of V.
  - fully-masked columns of near-diagonal tiles are skipped.
  - stage 2 is software-pipelined: PV matmuls trail their QK matmuls by
    LAG iterations so the PE never waits on the exp/bias of the tile it
    just produced; the pipeline runs flat across all (hp, qc, kb).
  - per-head-pair softmax reciprocal via reciprocal_approx_fast on DVE;
    1/den broadcast across partitions with tiny bf16 K=1 PE matmuls;
    normalization of head-pair hp is deferred into hp+1's stream.
  - oT / Wo in bf16 for a faster output projection.
"""
import sys

sys.path.insert(0, "/opt/trn_rl_repo")

import numpy as np
import ml_dtypes

import concourse.bass as bass
import concourse.mybir as mybir
import concourse.tile as tile
from concourse import bacc
from concourse.bass_utils import run_bass_kernel_spmd

F32, F32R, BF16 = mybir.dt.float32, mybir.dt.float32r, mybir.dt.bfloat16
AF = mybir.ActivationFunctionType
MULT = mybir.AluOpType.mult

B, S, E, H, D = 4, 2048, 1024, 16, 64
HL = H // 2          # heads per core
HD = HL * D          # 512, per-core head dims
NUM_BUCKETS, MAX_DISTANCE = 32, 128
NT = S // 128        # 16 token blocks
NE = E // 128        # 8 embed chunks
NFAR = 8             # far tiles only occur for kb < 8

_NC_CACHE = {}


# ---------------------------------------------------------------- host side

def _np_bucket(distance):
    """Mirror reference._relative_position_bucket for causal (distance>=0),
    float32 arithmetic like jnp."""
    max_exact = NUM_BUCKETS // 2  # 16
    is_small = distance < max_exact
    safe = np.maximum(distance, 1).astype(np.float32)
    log_scale = np.log(safe / np.float32(max_exact)).astype(np.float32) / np.float32(
        np.log(np.float32(MAX_DISTANCE / max_exact))
    )
    large = max_exact + (log_scale * np.float32(NUM_BUCKETS - max_exact)).astype(
        np.int32
    )
    large = np.minimum(large, NUM_BUCKETS - 1)
    return np.where(is_small, distance, large)


def _build_ebtab(rel_bias_half):
    """rel_bias_half [8, 32] -> ebtab [4 hp, 128 k, 8 mi, 2 h, 512 q] bf16
    holding exp(bias) * causal_mask (0/1). mi = (4*qc - kb) + 3 in [0, 8)."""
    qq = np.arange(512)[None, :]
    kk = np.arange(128)[:, None]
    eb = np.exp(rel_bias_half.astype(np.float32))  # [8, 32]
    tiles = []
    for mi in range(8):
        m = mi - 3
        dd = 128 * m + qq - kk  # q - k distance, [128, 512]
        bucket = _np_bucket(np.maximum(dd, 0))
        vals = eb[:, bucket]  # [8, 128, 512]
        vals = np.where(dd[None] >= 0, vals, 0.0).astype(np.float32)
        tiles.append(vals)
    t = np.stack(tiles, axis=0)  # [8 mi, 8 h, 128 k, 512 q]
    t = t.reshape(8, 4, 2, 128, 512).transpose(1, 3, 0, 2, 4)
    return np.ascontiguousarray(t).astype(ml_dtypes.bfloat16)


def _make_in_maps(inputs_q, inputs_kv, Wq, Wk, Wv, Wo, rel_bias):
    bf = ml_dtypes.bfloat16
    ebtabs = [_build_ebtab(rel_bias[0:HL]), _build_ebtab(rel_bias[HL:])]
    cvecs = []
    for half in range(2):
        c = np.exp(rel_bias[half * HL:(half + 1) * HL, 31].astype(np.float32))
        cv = np.broadcast_to(np.repeat(c, 65)[None, :], (128, HL * 65))
        cvecs.append(np.ascontiguousarray(cv).astype(bf))
    xqTs = [np.ascontiguousarray(inputs_q[b].T.astype(bf)) for b in range(B)]
    xkvTs = [np.ascontiguousarray(inputs_kv[b].T.astype(bf)) for b in range(B)]
    wqs, wks, wvs, wos = [], [], [], []
    for half in range(2):
        sl = slice(half * HD, (half + 1) * HD)
        wqs.append(np.ascontiguousarray(Wq[:, sl].astype(bf)))
        wks.append(np.ascontiguousarray(Wk[:, sl].astype(bf)))
        wvs.append(np.ascontiguousarray(Wv[:, sl].astype(bf)))
        wos.append(np.ascontiguousarray(Wo[sl, :].astype(bf)))
    in_maps = []
    for c in range(8):
        b, half = c // 2, c % 2
        in_maps.append({
            "xqT": xqTs[b],
            "xkvT": xkvTs[b],
            "wq": wqs[half],
            "wk": wks[half],
            "wv": wvs[half],
            "wo": wos[half],
            "ebtab": ebtabs[half],
            "cvec": cvecs[half],
        })
    return in_maps


# -------------------------------------------------------------- device side

def _build_nc():
    nc = bacc.Bacc(None, target_bir_lowering=False)
    xqT_d = nc.dram_tensor("xqT", [E, S], BF16, kind="ExternalInput")
    xkvT_d = nc.dram_tensor("xkvT", [E, S], BF16, kind="ExternalInput")
    wq_d = nc.dram_tensor("wq", [E, HD], BF16, kind="ExternalInput")
    wk_d = nc.dram_tensor("wk", [E, HD], BF16, kind="ExternalInput")
    wv_d = nc.dram_tensor("wv", [E, HD], BF16, kind="ExternalInput")
    wo_d = nc.dram_tensor("wo", [HD, E], BF16, kind="ExternalInput")
    eb_d = nc.dram_tensor("ebtab", [4, 128, 8, 2, 512], BF16,
                          kind="ExternalInput")
    cv_d = nc.dram_tensor("cvec", [128, HL * 65], BF16, kind="ExternalInput")
    out_d = nc.dram_tensor("out", [S, E], F32, kind="ExternalOutput")

    with tile.TileContext(nc) as tc:
        with (
            tc.tile_pool(name="const", bufs=1) as pconst,
            tc.tile_pool(name="persist", bufs=1) as pper,
        ):
            onesb = pconst.tile([128, 64], BF16)
            nc.vector.memset(onesb, 1.0)
            cvec = pconst.tile([128, HL * 65], BF16)
            nc.sync.dma_start(out=cvec, in_=cv_d[:])
            eb0 = pconst.tile([128, 8, 2, 512], BF16)
            nc.sync.dma_start(out=eb0, in_=eb_d[0])

            qT = pper.tile([128, 4, S], F32R)         # [pair-dims, hc, tok]
            kT = pper.tile([128, 4, S], F32R)
            vA = pper.tile([128, NT, HL * 65], BF16)  # v + ones col per head
            # vC only feeds far tiles (kb <= 4*qc-5 <= 7)
            vC = pper.tile([128, NFAR, HL * 65], BF16)  # vA * exp(b31_h)
            # per-head-pair denominator rows: head hh -> partition 32*hh
            den2 = pper.tile([128, 4, 512], F32)
            denRb = pper.tile([128, 4, 512], BF16)
            denS = pper.tile([128, 2, 32], F32)   # [64, head, 32] reshape

            vAr = vA.rearrange("p t (h c) -> p t h c", c=65)
            nc.vector.memset(vAr[:, :, :, 64:65], 1.0)

            # ---------------- stage 1: projections (x^T comes from host)
            with tc.tile_pool(name="s1x", bufs=1) as p1x:
                xkv_sb = p1x.tile([128, NE, S], BF16)
                xq_sb = p1x.tile([128, NE, S], BF16)

                with (
                    tc.tile_pool(name="s1wv", bufs=1) as p1wv,
                    tc.tile_pool(name="psPv", bufs=4, space="PSUM") as psPv,
                ):
                    wv_sb = p1wv.tile([128, NE, HD], BF16)
                    nc.sync.dma_start(
                        out=wv_sb,
                        in_=wv_d[:].rearrange("(c p) n -> p c n", p=128))
                    for hv in range(2):
                        for e in range(NE):
                            nc.sync.dma_start(
                                out=xkv_sb[:, e, hv * 1024:(hv + 1) * 1024],
                                in_=xkvT_d[e * 128:(e + 1) * 128,
                                           hv * 1024:(hv + 1) * 1024])
                    for hv in range(2):
                        for e in range(NE):
                            nc.sync.dma_start(
                                out=xq_sb[:, e, hv * 1024:(hv + 1) * 1024],
                                in_=xqT_d[e * 128:(e + 1) * 128,
                                          hv * 1024:(hv + 1) * 1024])
                    # v projection: out [tok, hd] blocks, evicted in pairs
                    for tp in range(NT // 2):
                        vps = psPv.tile([128, 2, HD], F32, tag="pj")
                        for ti in range(2):
                            t = 2 * tp + ti
                            for e in range(NE):
                                nc.tensor.matmul(
                                    vps[:, ti, :],
                                    xkv_sb[:, e, t * 128:(t + 1) * 128],
                                    wv_sb[:, e, :],
                                    start=(e == 0), stop=(e == NE - 1))
                        nc.vector.tensor_copy(
                            vAr[:, 2 * tp:2 * tp + 2, :, 0:64],
                            vps.rearrange("p a (h c) -> p a h c", c=64))
                    for t in range(NFAR):
                        nc.vector.tensor_tensor(
                            out=vC[:, t, :], in0=vA[:, t, :], in1=cvec,
                            op=MULT)

                # q/k projections: out qT/kT [2-head dims, tok]
                with (
                    tc.tile_pool(name="s1wqk", bufs=1) as p1w,
                    tc.tile_pool(name="psP", bufs=2, space="PSUM") as psP,
                ):
                    wk_sb = p1w.tile([128, NE, HD], BF16)
                    wq_sb = p1w.tile([128, NE, HD], BF16)
                    for w_sb, w_dr in ((wk_sb, wk_d), (wq_sb, wq_d)):
                        nc.sync.dma_start(
                            out=w_sb,
                            in_=w_dr[:].rearrange("(c p) n -> p c n", p=128))
                    for w_sb, x_sb, dst in ((wk_sb, xkv_sb, kT),
                                            (wq_sb, xq_sb, qT)):
                        for hc in range(4):
                            ps = psP.tile([128, 4, 512], F32, tag="pj")
                            for e in range(NE):
                                for tq in range(4):
                                    nc.tensor.matmul(
                                        ps[:, tq, :],
                                        w_sb[:, e, hc * 128:(hc + 1) * 128],
                                        x_sb[:, e, tq * 512:(tq + 1) * 512],
                                        start=(e == 0), stop=(e == NE - 1))
                            nc.vector.tensor_copy(
                                dst[:, hc, :],
                                ps.rearrange("p a b -> p (a b)"))

            # ---------------- stages 2+3 share the O^T pool
            with (
                tc.tile_pool(name="persist2", bufs=1) as pper2,
                tc.tile_pool(name="s3w", bufs=1) as p3w,
            ):
                oT = pper2.tile([128, 4, S], BF16)
                wo_sb = p3w.tile([128, 4, E], BF16)
                nc.sync.dma_start(
                    out=wo_sb,
                    in_=wo_d[:].rearrange("(g p) n -> p g n", p=128))

                with (
                    tc.tile_pool(name="s2eb", bufs=2) as p2b,
                    tc.tile_pool(name="s2p", bufs=6) as p2p,
                    tc.tile_pool(name="psS", bufs=3, space="PSUM") as psS,
                    tc.tile_pool(name="psO", bufs=1, space="PSUM") as psO,
                ):
                    eb_tiles = {0: eb0}
                    iters = [(hp, qc, kb)
                             for hp in range(4)
                             for qc in range(4)
                             for kb in range(4 * qc + 4)]
                    LAG = 3
                    p_info = {}
                    o_cur = [None]
                    norm_pending = []

                    def emit_norm(hp):
                        for qc in range(4):
                            pr = psS.tile([128, 512], F32, tag="s",
                                          name=f"pr{hp}_{qc}")
                            for hh in range(2):
                                bp = 32 * hh
                                nc.tensor.matmul(
                                    pr[64 * hh:64 * hh + 64, :],
                                    onesb[bp:bp + 1, 0:64],
                                    denRb[bp:bp + 1, qc, :],
                                    start=True, stop=True)
                            sl = oT[:, hp, qc * 512:(qc + 1) * 512]
                            nc.vector.tensor_tensor(
                                out=sl, in0=sl, in1=pr, op=MULT)

                    for j in range(len(iters) + LAG):
                        if j < len(iters):
                            hp, qc, kb = iters[j]
                            if qc == 1 and kb == 0 and hp < 3:
                                ebn = p2b.tile([128, 8, 2, 512], BF16,
                                               tag="eb", name=f"eb{hp + 1}")
                                nc.sync.dma_start(out=ebn, in_=eb_d[hp + 1])
                                eb_tiles[hp + 1] = ebn
                            m = 4 * qc - kb
                            q0 = 0 if m >= 0 else 128 * min(-m, 2)
                            s = psS.tile([128, 2, 512], F32, tag="s",
                                         name=f"s{j}")
                            nc.tensor.matmul(
                                s[:, 0, q0:512],
                                kT[0:64, hp, kb * 128:(kb + 1) * 128],
                                qT[0:64, hp, qc * 512 + q0:(qc + 1) * 512],
                                start=True, stop=True)
                            nc.tensor.matmul(
                                s[:, 1, q0:512],
                                kT[64:128, hp, kb * 128:(kb + 1) * 128],
                                qT[64:128, hp, qc * 512 + q0:(qc + 1) * 512],
                                start=True, stop=True)
                            p = p2p.tile([128, 2, 512], BF16, tag="p",
                                         name=f"p{j}")
                            nc.scalar.activation(
                                p[:, :, q0:512], s[:, :, q0:512],
                                AF.Exp, scale=0.125)
                            if m <= 4:  # near diagonal: elementwise bias
                                nc.vector.tensor_tensor(
                                    out=p[:, :, q0:512],
                                    in0=p[:, :, q0:512],
                                    in1=eb_tiles[hp][:, m + 3, :, q0:512],
                                    op=MULT)
                                p_info[j] = (p, q0, vA)
                            else:       # far: bias folded into vC
                                p_info[j] = (p, q0, vC)
                        jj = j - LAG
                        if jj >= 0:
                            hp, qc, kb = iters[jj]
                            h0, h1 = 2 * hp, 2 * hp + 1
                            p, q0, vsrc = p_info.pop(jj)
                            nkb = 4 * qc + 4
                            if kb == 0:
                                o_cur[0] = psO.tile([65, 2, 512], F32,
                                                    tag="o",
                                                    name=f"o{hp}_{qc}")
                            o = o_cur[0]
                            nc.tensor.matmul(
                                o[:, 0, q0:512],
                                vsrc[:, kb, h0 * 65:(h0 + 1) * 65],
                                p[:, 0, q0:512],
                                start=(kb == 0), stop=(kb == nkb - 1),
                                skip_group_check=True)
                            nc.tensor.matmul(
                                o[:, 1, q0:512],
                                vsrc[:, kb, h1 * 65:(h1 + 1) * 65],
                                p[:, 1, q0:512],
                                start=(kb == 0), stop=(kb == nkb - 1),
                                skip_group_check=True)
                            if kb == nkb - 1:
                                # epilogue: stash raw O^T + denominators
                                for hh in range(2):
                                    bp = 32 * hh
                                    nc.vector.tensor_copy(
                                        den2[bp:bp + 1, qc, :],
                                        o[64:65, hh, :])
                                    nc.vector.tensor_copy(
                                        oT[64 * hh:64 * (hh + 1), hp,
                                           qc * 512:(qc + 1) * 512],
                                        o[0:64, hh, :])
                                if qc == 3:
                                    for hh in range(2):
                                        bp = 32 * hh
                                        nc.sync.dma_start(
                                            out=denS[0:64, hh, :],
                                            in_=den2[bp:bp + 1, :, :])
                                        nc.vector.reciprocal(
                                            denS[0:64, hh, :],
                                            denS[0:64, hh, :])
                                        nc.sync.dma_start(
                                            out=den2[bp:bp + 1, :, :],
                                            in_=denS[0:64, hh, :])
                                        nc.vector.tensor_copy(
                                            denRb[bp:bp + 1, :, :],
                                            den2[bp:bp + 1, :, :])
                                    norm_pending.append(hp)
                                elif qc == 0 and norm_pending:
                                    emit_norm(norm_pending.pop(0))
                    while norm_pending:
                        emit_norm(norm_pending.pop(0))

                # ---------------- stage 3: output projection
                with (
                    tc.tile_pool(name="s3o", bufs=3) as p3o,
                    tc.tile_pool(name="psF", bufs=4, space="PSUM") as psF,
                ):
                    for t in range(NT):
                        oev = p3o.tile([128, E], F32, tag="oev")
                        ops = [psF.tile([128, 512], F32, tag="ops",
                                        name=f"ops{ec}")
                               for ec in range(2)]
                        for hp in range(4):
                            for ec in range(2):
                                nc.tensor.matmul(
                                    ops[ec], oT[:, hp, t * 128:(t + 1) * 128],
                                    wo_sb[:, hp, ec * 512:(ec + 1) * 512],
                                    start=(hp == 0), stop=(hp == 3))
                        for ec in range(2):
                            nc.vector.tensor_copy(
                                oev[:, ec * 512:(ec + 1) * 512], ops[ec])
                        nc.sync.dma_start(
                            out=out_d[t * 128:(t + 1) * 128, :], in_=oev)

    nc.compile()
    return nc


def _get_nc():
    if "nc" not in _NC_CACHE:
        _NC_CACHE["nc"] = _build_nc()
    return _NC_CACHE["nc"]


def kernel(inputs_q, inputs_kv, mask, Wq, Wk, Wv, Wo, rel_bias):
    inputs_q = np.asarray(inputs_q, dtype=np.float32)
    inputs_kv = np.asarray(inputs_kv, dtype=np.float32)
    Wq = np.asarray(Wq, dtype=np.float32)
    Wk = np.asarray(Wk, dtype=np.float32)
    Wv = np.asarray(Wv, dtype=np.float32)
    Wo = np.asarray(Wo, dtype=np.float32)
    rel_bias = np.asarray(rel_bias, dtype=np.float32)

    nc = _get_nc()
    in_maps = _make_in_maps(inputs_q, inputs_kv, Wq, Wk, Wv, Wo, rel_bias)
    res = run_bass_kernel_spmd(nc, in_maps, core_ids=list(range(8)))
    out = np.stack(
        [res.results[2 * b]["out"] + res.results[2 * b + 1]["out"]
         for b in range(B)])
    return out.astype(np.float32)
